# revision 32
# baseline (speedup 1.0000x reference)
"""BiLSTM single-step kernel for 8 Trainium2 NeuronCores.

Math per direction d (f, b):
    gates    = x_d @ Wx_d^T + h_d @ Wh_d^T + b_d          # [4096, 4*1024]
    f,i,o    = sigmoid(...), C = tanh(...)
    c_new    = f*c + i*C ; h_new = o*tanh(c_new)

Distribution: data-parallel over batch, 512 rows per core; weights
replicated. Per core each direction is a [512, 2048] x [2048, 4096] GEMM.

Precision strategy: the x-part (|x|~1) runs in fp16 for the C/i/o gates;
the h-part is tiny (|h|~0.02, |h.Wh| ~ 2% of the gate magnitude) and runs
in fp8-e5m2 with DoubleRow perf mode (2 k-chunks per matmul instruction),
accumulating into the same fp32 PSUM bank. The f gate's x-part ALSO runs
in fp8-e5m2 DoubleRow: its gate output only multiplies the tiny cell
state (|c| <= ~0.1), so its error sensitivity is ~10x lower than the
other gates (CPU-verified end-to-end relmax ~5e-3, vs the 2e-2 budget).

On-chip layout is the transpose of the reference: psum tiles are
gates^T [128 gate-hidden partitions, 512 batch], so the per-(gate,h) bias
is per-partition (fused into the scalar-engine sigmoid/tanh) and the
contraction index i sits on SBUF partitions for both matmul operands.
All transposes happen host-side in numpy.

Startup schedule (from perfetto analysis of earlier revisions): the DMA
rings take ~9 us to start flowing and are per-transfer latency-bound
while ramping, so group 0 (which needs all of d0's activations plus its
weights, ~3.4 MB, vs ~9.5 us of matmul time) is inherently DMA-starved.
The fix is to front-load the DMA-cheapest real work:
  - ~14 dummy matmuls on a zeroed tile run during the pure-DMA fill so
    the PE_HAM clock gate is warm (2.4 GHz) when real matmuls start,
    and a few more are interleaved at known seams (an idle 3.4 us HAM
    window would re-throttle the PE to 1.2 GHz).
  - The fp8 h-part matmuls of groups 0 AND 1 run first (28 DoubleRow
    matmuls off ~1.5 MB of fp8 data), holding 7 open PSUM accumulations,
    j-major so each arriving combh slice feeds four matmuls.
  - The fp16 x-phases then run k-major (three matmuls per arriving
    combx chunk, in arrival order).
  - fp16 x-weights alternate between the gpsimd and sync rings; the
    final groups pin to sync so the tail stores don't queue behind
    weight traffic on gpsimd.
"""

import numpy as np
import ml_dtypes

import concourse.bass as bass
import concourse.mybir as mybir
import concourse.tile as tile
from concourse import bacc, bass_utils
from concourse.bass import ts

BATCH, IN, HID = 4096, 1024, 1024
NCORES = 8
BS = BATCH // NCORES          # 512 batch rows per core = matmul free dim N
KX = IN // 128                # 8 contraction chunks (x part)
KH = HID // 128               # 8 fp8 contraction chunks (h part)
HC = HID // 128               # 8 hidden chunks of 128

F16 = mybir.dt.float16
F8 = mybir.dt.float8e5
F32 = mybir.dt.float32
AF = mybir.ActivationFunctionType
DR = mybir.MatmulPerfMode.DoubleRow

GPERM = (3, 0, 1, 2)  # gate consumption order (tanh gate first)

# Stashed by kernel() so a test harness can read exec_time_ns / trace paths.
LAST_RESULTS = None


def _build_nc():
    nc = bacc.Bacc("TRN2", target_bir_lowering=False, debug=False,
                   num_devices=NCORES)

    combx_d = nc.dram_tensor("combx", [2, 128, KX * BS], F16,
                             kind="ExternalInput").ap()
    combh_d = nc.dram_tensor("combh", [2, 128, KH, BS], F8,
                             kind="ExternalInput").ap()
    combx8_d = nc.dram_tensor("combx8", [2, 128, KX, BS], F8,
                              kind="ExternalInput").ap()
    # g dimension pre-permuted host-side into consumption order (3,0,1,2).
    wx_d = nc.dram_tensor("wx", [2, HC, 4, 128, KX * 128], F16,
                          kind="ExternalInput").ap()
    wx8_d = nc.dram_tensor("wx8", [2, HC, 128, KX, 128], F8,
                           kind="ExternalInput").ap()
    wh_d = nc.dram_tensor("wh", [2, HC, 128, 4, KH, 128], F8,
                          kind="ExternalInput").ap()
    ct_d = nc.dram_tensor("ct", [2, HC, 128, BS], F32,
                          kind="ExternalInput").ap()
    bias_d = nc.dram_tensor("bias", [2, 128, 4 * HC], F32,
                            kind="ExternalInput").ap()
    hT_d = nc.dram_tensor("hT", [2, HC, 128, BS], F32,
                          kind="ExternalOutput").ap()
    cT_d = nc.dram_tensor("cT", [2, HC, 128, BS], F32,
                          kind="ExternalOutput").ap()

    with tile.TileContext(nc) as tc:
        with (
            tc.tile_pool(name="comb", bufs=2) as comb_pool,
            tc.tile_pool(name="w", bufs=8) as w_pool,
            tc.tile_pool(name="w16", bufs=12) as w16_pool,
            tc.tile_pool(name="psum", bufs=6, space="PSUM") as psum_pool,
            tc.tile_pool(name="dpsum", bufs=2, space="PSUM") as dummy_pool,
            tc.tile_pool(name="gates", bufs=8) as gate_pool,
            tc.tile_pool(name="cc", bufs=3) as c_pool,
            tc.tile_pool(name="tmp", bufs=3) as tmp_pool,
            tc.tile_pool(name="biasp", bufs=2) as bias_pool,
        ):
            warm = comb_pool.tile([128, BS], F16, name="warm", tag="warm")
            nc.gpsimd.memset(warm[:], 0.0)

            def dummy_mms(n):
                for _ in range(n):
                    wps = dummy_pool.tile([128, BS], F32, name="wps",
                                          tag="wps")
                    nc.tensor.matmul(wps[:], warm[:, :128], warm[:],
                                     start=True, stop=True)

            # HAM warmup during the initial DMA fill; sized so the dummy
            # stream ends right as the first real matmul's data lands
            # (~8 run at the cold 1.2 GHz clock, the rest warm).
            dummy_mms(14)

            state = {}

            def load_group(d, hc, combx1_after=False):
                """Emit the weight (and ct) DMAs for one (d, hc) group."""
                first = d == 0 and hc == 0
                wt8 = w_pool.tile([128, 4, KH, 128], F8,
                                  name="wt8", tag="wt8")
                if first or (d == 0 and hc == 1):
                    # split so the startup h-quads gate on 128 KB slices
                    nc.sync.dma_start(wt8[:, 0], wh_d[d, hc, :, 0])
                    nc.sync.dma_start(wt8[:, 1], wh_d[d, hc, :, 1])
                    nc.sync.dma_start(wt8[:, 2:], wh_d[d, hc, :, 2:])
                else:
                    nc.sync.dma_start(wt8[:], wh_d[d, hc])
                wt8x = w_pool.tile([128, KX, 128], F8,
                                   name="wt8x", tag="wt8x")
                nc.sync.dma_start(wt8x[:], wx8_d[d, hc])
                ct = c_pool.tile([128, BS], F32, name="ct_t", tag="ct_t")
                if not first:
                    nc.gpsimd.dma_start(ct[:], ct_d[d, hc])
                if d == 1 and hc >= HC - 2:
                    w16_eng = nc.sync
                else:
                    w16_eng = nc.gpsimd if hc % 2 == 0 else nc.sync
                wts = {}
                for gi, g in enumerate(GPERM):
                    if g == 0:
                        continue
                    wt = w16_pool.tile([128, KX * 128], F16,
                                       name="wt", tag="wt")
                    w16_eng.dma_start(wt[:], wx_d[d, hc, gi])
                    wts[g] = wt
                if first:
                    # gpsimd queue: k0, fp16 weights, second combx half,
                    # ct - matching the k-major consumption order.
                    nc.gpsimd.dma_start(state["combxs"][1][:],
                                        combx_d[d, :, 4 * BS:8 * BS])
                    nc.gpsimd.dma_start(ct[:], ct_d[d, hc])
                return {"wt8": wt8, "wt8x": wt8x, "wts": wts, "ct": ct}

            def h_quad(ps, gi, wt8, start):
                for j in range(KH // 2):
                    nc.tensor.matmul(
                        ps[:], wt8[:, gi, 2 * j:2 * j + 2, :],
                        state["combh"][:, 2 * j:2 * j + 2, :],
                        start=(start and j == 0), stop=False,
                        perf_mode=DR,
                    )

            def fx_mms(ps, wt8x, seam=0):
                for j in range(KX // 2):
                    nc.tensor.matmul(
                        ps[:], wt8x[:, 2 * j:2 * j + 2, :],
                        state["combx8"][:, 2 * j:2 * j + 2, :],
                        start=False, stop=(j == KX // 2 - 1),
                        perf_mode=DR,
                    )
                    if seam and j == 1:
                        dummy_mms(seam)

            def x_kmajor(pss, wts):
                for k in range(KX):
                    for g in (3, 1, 2):
                        nc.tensor.matmul(
                            pss[g][:], wts[g][:, ts(k, 128)],
                            state["combxs"][k // 4][:, ts(k % 4, BS)],
                            start=False, stop=(k == KX - 1),
                        )

            def x_gate(ps, wt):
                for k in range(KX):
                    nc.tensor.matmul(
                        ps[:], wt[:, ts(k, 128)],
                        state["combxs"][k // 4][:, ts(k % 4, BS)],
                        start=False, stop=(k == KX - 1),
                    )

            def act_gate(d, hc, g, src, n0=0, n1=BS):
                gt = gate_pool.tile([128, n1 - n0], F32, name="gt",
                                    tag="gt")
                nc.scalar.activation(
                    gt[:], src[:, n0:n1] if (n0, n1) != (0, BS) else src[:],
                    AF.Sigmoid if g < 3 else AF.Tanh,
                    bias=state["bias_t"][:, g * HC + hc: g * HC + hc + 1],
                )
                return gt

            def fuse_store(d, hc, gts, ct):
                """Elementwise gate fusion + output stores for one group."""
                last = d == 1 and hc == HC - 1
                t1 = tmp_pool.tile([128, BS], F32, name="t1", tag="t1")
                nc.vector.tensor_mul(t1[:], gts[0][:], ct[:])
                t2 = tmp_pool.tile([128, BS], F32, name="t2", tag="t2")
                nc.vector.tensor_mul(t2[:], gts[1][:], gts[3][:])
                cnew = tmp_pool.tile([128, BS], F32, name="cnew",
                                     tag="cnew")
                nc.vector.tensor_add(cnew[:], t1[:], t2[:])
                tanhc = tmp_pool.tile([128, BS], F32, name="tanhc",
                                      tag="tanhc")
                nc.scalar.activation(tanhc[:], cnew[:], AF.Tanh)
                if last:
                    # tail: halves on two rings so the final stores drain
                    # in parallel instead of serializing.
                    HB = BS // 2
                    nc.sync.dma_start(cT_d[d, hc, :, :HB], cnew[:, :HB])
                    nc.gpsimd.dma_start(cT_d[d, hc, :, HB:], cnew[:, HB:])
                else:
                    nc.scalar.dma_start(cT_d[d, hc], cnew[:])
                if isinstance(gts[2], list):
                    HB = BS // 2
                    store_eng = (nc.scalar, nc.sync)
                    for h2, oH in enumerate(gts[2]):
                        hnH = tmp_pool.tile([128, HB], F32,
                                            name="hnH", tag="hnew")
                        nc.vector.tensor_mul(
                            hnH[:], oH[:], tanhc[:, h2 * HB:(h2 + 1) * HB])
                        store_eng[h2].dma_start(
                            hT_d[d, hc, :, h2 * HB:(h2 + 1) * HB], hnH[:])
                else:
                    hnew = tmp_pool.tile([128, BS], F32, name="hnew",
                                         tag="hnew")
                    nc.vector.tensor_mul(hnew[:], gts[2][:], tanhc[:])
                    nc.scalar.dma_start(hT_d[d, hc], hnew[:])

            def steady_group(d, hc, grp):
                """One full group: all four gates, then fusion."""
                wt8, wt8x, wts = grp["wt8"], grp["wt8x"], grp["wts"]
                gts = {}
                for gi, g in enumerate(GPERM):
                    if d == 1 and hc == HC - 1 and gi == 3:
                        # Final group: two half-N chains so the first
                        # half's ACT/DVE/store pipeline under the second
                        # half's matmuls.
                        halves = []
                        HB = BS // 2
                        for h2 in range(2):
                            psH = psum_pool.tile([128, HB], F32,
                                                 name="psH", tag="ps")
                            for j in range(KH // 2):
                                nc.tensor.matmul(
                                    psH[:], wt8[:, gi, 2 * j:2 * j + 2, :],
                                    state["combh"][:, 2 * j:2 * j + 2,
                                                   h2 * HB:(h2 + 1) * HB],
                                    start=(j == 0), stop=False,
                                    perf_mode=DR,
                                )
                            for k in range(KX):
                                base = (k % 4) * BS + h2 * HB
                                nc.tensor.matmul(
                                    psH[:], wts[g][:, ts(k, 128)],
                                    state["combxs"][k // 4][:,
                                                            base:base + HB],
                                    start=False, stop=(k == KX - 1),
                                )
                            gtH = gate_pool.tile([128, HB], F32,
                                                 name="gtH", tag="gt")
                            nc.scalar.activation(
                                gtH[:], psH[:], AF.Sigmoid,
                                bias=state["bias_t"][:, g * HC + hc:
                                                     g * HC + hc + 1],
                            )
                            halves.append(gtH)
                        gts[g] = halves
                        continue
                    ps = psum_pool.tile([128, BS], F32, name="ps",
                                        tag="ps")
                    h_quad(ps, gi, wt8, start=True)
                    if g == 0:
                        fx_mms(ps, wt8x)
                    else:
                        x_gate(ps, wts[g])
                    gts[g] = act_gate(d, hc, g, ps)
                gts = [gts[0], gts[1], gts[2], gts[3]]
                fuse_store(d, hc, gts, grp["ct"])

            for d in range(2):
                combh = comb_pool.tile([128, KH, BS], F8, name="combh",
                                       tag="combh")
                state["combh"] = combh
                if d == 0:
                    # three slices matching the j-major startup schedule.
                    nc.scalar.dma_start(combh[:, :2, :],
                                        combh_d[d, :, :2, :])
                    nc.scalar.dma_start(combh[:, 2:4, :],
                                        combh_d[d, :, 2:4, :])
                    nc.scalar.dma_start(combh[:, 4:, :],
                                        combh_d[d, :, 4:, :])
                else:
                    nc.scalar.dma_start(combh[:, :KH // 2, :],
                                        combh_d[d, :, :KH // 2, :])
                    nc.scalar.dma_start(combh[:, KH // 2:, :],
                                        combh_d[d, :, KH // 2:, :])
                combx8 = comb_pool.tile([128, KX, BS], F8, name="combx8",
                                        tag="combx8")
                state["combx8"] = combx8
                if d == 0:
                    nc.scalar.dma_start(combx8[:, :4, :],
                                        combx8_d[d, :, :4, :])
                    nc.scalar.dma_start(combx8[:, 4:, :],
                                        combx8_d[d, :, 4:, :])
                else:
                    nc.gpsimd.dma_start(combx8[:], combx8_d[d])
                bias_t = bias_pool.tile([128, 4 * HC], F32, name="bias_t",
                                        tag="bias_t")
                state["bias_t"] = bias_t
                nc.gpsimd.dma_start(bias_t[:], bias_d[d])
                combxs = []
                for cc in range(2):
                    cb = comb_pool.tile([128, 4 * BS], F16,
                                        name=f"combx{cc}", tag=f"combx{cc}")
                    if d == 0 and cc == 0:
                        nc.gpsimd.dma_start(cb[:, :BS],
                                            combx_d[d, :, :BS])
                        nc.scalar.dma_start(cb[:, BS:],
                                            combx_d[d, :, BS:4 * BS])
                    elif d == 0:
                        pass  # issued inside load_group(0, 0)
                    else:
                        nc.scalar.dma_start(cb[:],
                                            combx_d[d, :, ts(cc, 4 * BS)])
                    combxs.append(cb)
                state["combxs"] = combxs

                if d == 0:
                    # ---- startup: groups 0 and 1 interleaved ----------
                    g0 = load_group(0, 0)
                    g1 = load_group(0, 1)
                    # h-parts of group 0, j-major with dummy seams.
                    pss0 = {g: psum_pool.tile([128, BS], F32, name="ps",
                                              tag="ps") for g in GPERM}
                    for j in range(KH // 2):
                        for gi, g in enumerate(GPERM):
                            nc.tensor.matmul(
                                pss0[g][:],
                                g0["wt8"][:, gi, 2 * j:2 * j + 2, :],
                                combh[:, 2 * j:2 * j + 2, :],
                                start=(j == 0), stop=False, perf_mode=DR,
                            )
                        if j == 0:
                            dummy_mms(2)
                        elif j == 1:
                            dummy_mms(4)
                    # f gate x-part of group 0 (fp8, arrives early).
                    fx_mms(pss0[0], g0["wt8x"], seam=2)
                    # h-parts of group 1's C/f gates: more DMA-cheap
                    # work while the fp16 stream ramps (2 more open
                    # psum banks; i/o quads run after group 0 retires).
                    pss1 = {}
                    for gi, g in [(0, 3), (1, 0)]:
                        pss1[g] = psum_pool.tile([128, BS], F32,
                                                 name="ps", tag="ps")
                        h_quad(pss1[g], gi, g1["wt8"], start=True)
                    # group 0's fp16 x-phase, k-major.
                    x_kmajor(pss0, g0["wts"])
                    gts0 = {}
                    for g in (0, 3, 1, 2):
                        gts0[g] = act_gate(0, 0, g, pss0[g])
                    fuse_store(0, 0, [gts0[0], gts0[1], gts0[2], gts0[3]],
                               g0["ct"])
                    # group 1: remaining work.
                    fx_mms(pss1[0], g1["wt8x"])
                    for gi, g in [(2, 1), (3, 2)]:
                        pss1[g] = psum_pool.tile([128, BS], F32,
                                                 name="ps", tag="ps")
                        h_quad(pss1[g], gi, g1["wt8"], start=True)
                    x_kmajor(pss1, g1["wts"])
                    gts1 = {}
                    for g in (0, 3, 1, 2):
                        gts1[g] = act_gate(0, 1, g, pss1[g])
                    fuse_store(0, 1, [gts1[0], gts1[1], gts1[2], gts1[3]],
                               g1["ct"])
                    hc_start = 2
                else:
                    hc_start = 0
                for hc in range(hc_start, HC):
                    grp = load_group(d, hc)
                    steady_group(d, hc, grp)
    nc.compile()
    return nc


def _prep_w(W):
    # W [4, 1024, 2048] f32 (gate, h, i) -> (wx fp16, wx8 fp8, wh fp8):
    # wx  [HC, 4(perm), 128 i_local, KX*128 (k, h_local)] from i in [0, 1024)
    # wx8 [HC, 128 i_local, KX, 128 h_local]  f-gate slice of the same range
    # wh  [HC, 128 i_local, 4(perm), KH, 128 h_local]  from i in [1024, 2048)
    # so the lhsT tile for (gate, hc, k) has i on partitions, with the gate
    # dim pre-permuted to the kernel's consumption order.
    w5 = W.reshape(4, HC, 128, 16, 128).transpose(0, 1, 4, 3, 2)[list(GPERM)]
    # w5: [g(perm), hc, i_local, k(0..15), h_local]
    wx = np.ascontiguousarray(
        w5[:, :, :, :KX, :].transpose(1, 0, 2, 3, 4)
    ).astype(np.float16).reshape(HC, 4, 128, KX * 128)
    # f gate sits at permuted slot 1 (GPERM.index of gate 0)
    wx8 = np.ascontiguousarray(
        w5[1, :, :, :KX, :]
    ).astype(ml_dtypes.float8_e5m2)
    wh = np.ascontiguousarray(
        w5[:, :, :, KX:, :].transpose(1, 2, 0, 3, 4)
    ).astype(ml_dtypes.float8_e5m2)
    return wx, wx8, wh


def _prep_combx(x_slice):
    # [BS, 1024] f16 -> [128 i_local, KX*BS (k, b)]
    return np.ascontiguousarray(
        x_slice.T.reshape(KX, 128, BS).transpose(1, 0, 2)
    ).reshape(128, KX * BS)


def _prep_comb8(x_slice):
    # [BS, 1024] f32 -> fp8 [128 i_local, K, BS]
    return np.ascontiguousarray(
        x_slice.T.reshape(KX, 128, BS).transpose(1, 0, 2)
    ).astype(ml_dtypes.float8_e5m2)


def _prep_ct(c_slice):
    # [BS, 1024] f32 -> [HC, 128 h_local, BS]
    return np.ascontiguousarray(c_slice.T).reshape(HC, 128, BS)


def _prep_bias(b):
    # [4, 1024] f32 -> [128 h_local, 4*HC (g, hc)]
    return np.ascontiguousarray(
        b.reshape(4, HC, 128).transpose(2, 0, 1)
    ).reshape(128, 4 * HC)


def kernel(input_f, input_b, Hidden_State_f, Cell_State_f,
           Hidden_State_b, Cell_State_b, Wf, bf, Wb, bb):
    global LAST_RESULTS

    args = [np.asarray(a, dtype=np.float32) for a in (
        input_f, input_b, Hidden_State_f, Cell_State_f,
        Hidden_State_b, Cell_State_b, Wf, bf, Wb, bb)]
    (input_f, input_b, Hidden_State_f, Cell_State_f,
     Hidden_State_b, Cell_State_b, Wf, bf, Wb, bb) = args

    xf16 = input_f.astype(np.float16)
    xb16 = input_b.astype(np.float16)
    wxf, wx8f, whf = _prep_w(Wf)
    wxb, wx8b, whb = _prep_w(Wb)
    wx_all = np.stack([wxf, wxb])
    wx8_all = np.stack([wx8f, wx8b])
    wh_all = np.stack([whf, whb])
    bias_all = np.stack([_prep_bias(bf), _prep_bias(bb)])

    in_maps = []
    for c in range(NCORES):
        sl = slice(c * BS, (c + 1) * BS)
        in_maps.append({
            "combx": np.stack([_prep_combx(xf16[sl]), _prep_combx(xb16[sl])]),
            "combx8": np.stack([_prep_comb8(input_f[sl]),
                                _prep_comb8(input_b[sl])]),
            "combh": np.stack([_prep_comb8(Hidden_State_f[sl]),
                               _prep_comb8(Hidden_State_b[sl])]),
            "wx": wx_all,
            "wx8": wx8_all,
            "wh": wh_all,
            "ct": np.stack([_prep_ct(Cell_State_f[sl]),
                            _prep_ct(Cell_State_b[sl])]),
            "bias": bias_all,
        })

    nc = _build_nc()
    res = bass_utils.run_bass_kernel_spmd(nc, in_maps,
                                          core_ids=list(range(NCORES)))
    LAST_RESULTS = res

    h_f = np.empty((BATCH, HID), np.float32)
    c_f = np.empty((BATCH, HID), np.float32)
    h_b = np.empty((BATCH, HID), np.float32)
    c_b = np.empty((BATCH, HID), np.float32)
    for c in range(NCORES):
        sl = slice(c * BS, (c + 1) * BS)
        r = res.results[c]
        hT, cT = r["hT"], r["cT"]  # [2, HC, 128, BS] f32
        h_f[sl] = hT[0].reshape(HID, BS).T
        c_f[sl] = cT[0].reshape(HID, BS).T
        h_b[sl] = hT[1].reshape(HID, BS).T
        c_b[sl] = cT[1].reshape(HID, BS).T
    return h_f, c_f, h_b, c_b


# revision 34
# speedup vs baseline: 1.0103x; 1.0103x over previous
"""BiLSTM single-step kernel for 8 Trainium2 NeuronCores.

Math per direction d (f, b):
    gates    = x_d @ Wx_d^T + h_d @ Wh_d^T + b_d          # [4096, 4*1024]
    f,i,o    = sigmoid(...), C = tanh(...)
    c_new    = f*c + i*C ; h_new = o*tanh(c_new)

Distribution: data-parallel over batch, 512 rows per core; weights
replicated. Per core each direction is a [512, 2048] x [2048, 4096] GEMM.

Precision strategy: the x-part (|x|~1) runs in fp16 for the C/i/o gates;
the h-part is tiny (|h|~0.02, |h.Wh| ~ 2% of the gate magnitude) and runs
in fp8-e5m2 with DoubleRow perf mode (2 k-chunks per matmul instruction),
accumulating into the same fp32 PSUM bank. The f gate's x-part ALSO runs
in fp8-e5m2 DoubleRow: its gate output only multiplies the tiny cell
state (|c| <= ~0.1), so its error sensitivity is ~10x lower than the
other gates (CPU-verified end-to-end relmax ~5e-3, vs the 2e-2 budget).

On-chip layout is the transpose of the reference: psum tiles are
gates^T [128 gate-hidden partitions, 512 batch], so the per-(gate,h) bias
is per-partition (fused into the scalar-engine sigmoid/tanh) and the
contraction index i sits on SBUF partitions for both matmul operands.
All transposes happen host-side in numpy.

Startup schedule (from perfetto analysis of earlier revisions): the DMA
rings take ~9 us to start flowing and are per-transfer latency-bound
while ramping, so group 0 (which needs all of d0's activations plus its
weights, ~3.4 MB, vs ~9.5 us of matmul time) is inherently DMA-starved.
The fix is to front-load the DMA-cheapest real work:
  - ~14 dummy matmuls on a zeroed tile run during the pure-DMA fill so
    the PE_HAM clock gate is warm (2.4 GHz) when real matmuls start,
    and a few more are interleaved at known seams (an idle 3.4 us HAM
    window would re-throttle the PE to 1.2 GHz).
  - The fp8 h-part matmuls of groups 0 AND 1 run first (28 DoubleRow
    matmuls off ~1.5 MB of fp8 data), holding 7 open PSUM accumulations,
    j-major so each arriving combh slice feeds four matmuls.
  - The fp16 x-phases then run k-major (three matmuls per arriving
    combx chunk, in arrival order).
  - fp16 x-weights alternate between the gpsimd and sync rings; the
    final groups pin to sync so the tail stores don't queue behind
    weight traffic on gpsimd.
"""

import numpy as np
import ml_dtypes

import concourse.bass as bass
import concourse.mybir as mybir
import concourse.tile as tile
from concourse import bacc, bass_utils
from concourse.bass import ts

BATCH, IN, HID = 4096, 1024, 1024
NCORES = 8
BS = BATCH // NCORES          # 512 batch rows per core = matmul free dim N
KX = IN // 128                # 8 contraction chunks (x part)
KH = HID // 128               # 8 fp8 contraction chunks (h part)
HC = HID // 128               # 8 hidden chunks of 128

F16 = mybir.dt.float16
F8 = mybir.dt.float8e5
F32 = mybir.dt.float32
AF = mybir.ActivationFunctionType
DR = mybir.MatmulPerfMode.DoubleRow

GPERM = (3, 0, 1, 2)  # gate consumption order (tanh gate first)

# Stashed by kernel() so a test harness can read exec_time_ns / trace paths.
LAST_RESULTS = None


def _build_nc():
    nc = bacc.Bacc("TRN2", target_bir_lowering=False, debug=False,
                   num_devices=NCORES)

    combx_d = nc.dram_tensor("combx", [2, 128, KX * BS], F16,
                             kind="ExternalInput").ap()
    combh_d = nc.dram_tensor("combh", [2, 128, KH, BS], F8,
                             kind="ExternalInput").ap()
    combx8_d = nc.dram_tensor("combx8", [2, 128, KX, BS], F8,
                              kind="ExternalInput").ap()
    # g dimension pre-permuted host-side into consumption order (3,0,1,2).
    wx_d = nc.dram_tensor("wx", [2, HC, 4, 128, KX * 128], F16,
                          kind="ExternalInput").ap()
    wx8_d = nc.dram_tensor("wx8", [2, HC, 128, KX, 128], F8,
                           kind="ExternalInput").ap()
    wh_d = nc.dram_tensor("wh", [2, HC, 128, 4, KH, 128], F8,
                          kind="ExternalInput").ap()
    ct_d = nc.dram_tensor("ct", [2, HC, 128, BS], F32,
                          kind="ExternalInput").ap()
    bias_d = nc.dram_tensor("bias", [2, 128, 4 * HC], F32,
                            kind="ExternalInput").ap()
    hT_d = nc.dram_tensor("hT", [2, HC, 128, BS], F32,
                          kind="ExternalOutput").ap()
    cT_d = nc.dram_tensor("cT", [2, HC, 128, BS], F32,
                          kind="ExternalOutput").ap()

    with tile.TileContext(nc) as tc:
        with (
            tc.tile_pool(name="comb", bufs=2) as comb_pool,
            tc.tile_pool(name="w", bufs=8) as w_pool,
            tc.tile_pool(name="w16", bufs=12) as w16_pool,
            tc.tile_pool(name="psum", bufs=6, space="PSUM") as psum_pool,
            tc.tile_pool(name="dpsum", bufs=2, space="PSUM") as dummy_pool,
            tc.tile_pool(name="gates", bufs=8) as gate_pool,
            tc.tile_pool(name="cc", bufs=3) as c_pool,
            tc.tile_pool(name="tmp", bufs=3) as tmp_pool,
            tc.tile_pool(name="biasp", bufs=2) as bias_pool,
        ):
            # fp8 DoubleRow dummies: the weight-load path pays ~0.5 us to
            # switch between fp16-FWL and DoubleRow configs, so dummies
            # match the DR phases they sit next to.
            warm = comb_pool.tile([128, 2, BS], F8, name="warm",
                                  tag="warm")
            nc.gpsimd.memset(warm[:], 0.0)

            def dummy_mms(n):
                for _ in range(n):
                    wps = dummy_pool.tile([128, BS], F32, name="wps",
                                          tag="wps")
                    nc.tensor.matmul(wps[:], warm[:, :, :128], warm[:],
                                     start=True, stop=True,
                                     perf_mode=DR)

            # HAM warmup during the initial DMA fill; sized so the dummy
            # stream ends right as the first real matmul's data lands
            # (~8 run at the cold 1.2 GHz clock, the rest warm).
            dummy_mms(14)

            state = {}

            def load_group(d, hc, combx1_after=False):
                """Emit the weight (and ct) DMAs for one (d, hc) group."""
                first = d == 0 and hc == 0
                wt8 = w_pool.tile([128, 4, KH, 128], F8,
                                  name="wt8", tag="wt8")
                if first or (d == 0 and hc == 1):
                    # split so the startup h-quads gate on 128 KB slices
                    nc.sync.dma_start(wt8[:, 0], wh_d[d, hc, :, 0])
                    nc.sync.dma_start(wt8[:, 1], wh_d[d, hc, :, 1])
                    nc.sync.dma_start(wt8[:, 2:], wh_d[d, hc, :, 2:])
                else:
                    nc.sync.dma_start(wt8[:], wh_d[d, hc])
                wt8x = w_pool.tile([128, KX, 128], F8,
                                   name="wt8x", tag="wt8x")
                nc.sync.dma_start(wt8x[:], wx8_d[d, hc])
                ct = c_pool.tile([128, BS], F32, name="ct_t", tag="ct_t")
                if not first:
                    nc.gpsimd.dma_start(ct[:], ct_d[d, hc])
                if d == 1 and hc >= HC - 2:
                    w16_eng = nc.sync
                else:
                    w16_eng = nc.gpsimd if hc % 2 == 0 else nc.sync
                wts = {}
                for gi, g in enumerate(GPERM):
                    if g == 0:
                        continue
                    wt = w16_pool.tile([128, KX * 128], F16,
                                       name="wt", tag="wt")
                    w16_eng.dma_start(wt[:], wx_d[d, hc, gi])
                    wts[g] = wt
                if first:
                    # gpsimd queue: k0, fp16 weights, second combx half,
                    # ct - matching the k-major consumption order.
                    nc.gpsimd.dma_start(state["combxs"][1][:],
                                        combx_d[d, :, 4 * BS:8 * BS])
                    nc.gpsimd.dma_start(ct[:], ct_d[d, hc])
                return {"wt8": wt8, "wt8x": wt8x, "wts": wts, "ct": ct}

            def h_quad(ps, gi, wt8, start):
                for j in range(KH // 2):
                    nc.tensor.matmul(
                        ps[:], wt8[:, gi, 2 * j:2 * j + 2, :],
                        state["combh"][:, 2 * j:2 * j + 2, :],
                        start=(start and j == 0), stop=False,
                        perf_mode=DR,
                    )

            def fx_mms(ps, wt8x, seam=0):
                for j in range(KX // 2):
                    nc.tensor.matmul(
                        ps[:], wt8x[:, 2 * j:2 * j + 2, :],
                        state["combx8"][:, 2 * j:2 * j + 2, :],
                        start=False, stop=(j == KX // 2 - 1),
                        perf_mode=DR,
                    )
                    if seam and j == 1:
                        dummy_mms(seam)

            def x_kmajor(pss, wts):
                for k in range(KX):
                    for g in (3, 1, 2):
                        nc.tensor.matmul(
                            pss[g][:], wts[g][:, ts(k, 128)],
                            state["combxs"][k // 4][:, ts(k % 4, BS)],
                            start=False, stop=(k == KX - 1),
                        )

            def x_gate(ps, wt):
                for k in range(KX):
                    nc.tensor.matmul(
                        ps[:], wt[:, ts(k, 128)],
                        state["combxs"][k // 4][:, ts(k % 4, BS)],
                        start=False, stop=(k == KX - 1),
                    )

            def act_gate(d, hc, g, src, n0=0, n1=BS):
                gt = gate_pool.tile([128, n1 - n0], F32, name="gt",
                                    tag="gt")
                nc.scalar.activation(
                    gt[:], src[:, n0:n1] if (n0, n1) != (0, BS) else src[:],
                    AF.Sigmoid if g < 3 else AF.Tanh,
                    bias=state["bias_t"][:, g * HC + hc: g * HC + hc + 1],
                )
                return gt

            def fuse_store(d, hc, gts, ct):
                """Elementwise gate fusion + output stores for one group."""
                last = d == 1 and hc == HC - 1
                t1 = tmp_pool.tile([128, BS], F32, name="t1", tag="t1")
                nc.vector.tensor_mul(t1[:], gts[0][:], ct[:])
                t2 = tmp_pool.tile([128, BS], F32, name="t2", tag="t2")
                nc.vector.tensor_mul(t2[:], gts[1][:], gts[3][:])
                cnew = tmp_pool.tile([128, BS], F32, name="cnew",
                                     tag="cnew")
                nc.vector.tensor_add(cnew[:], t1[:], t2[:])
                tanhc = tmp_pool.tile([128, BS], F32, name="tanhc",
                                      tag="tanhc")
                nc.scalar.activation(tanhc[:], cnew[:], AF.Tanh)
                if last:
                    # tail: halves on two rings so the final stores drain
                    # in parallel instead of serializing.
                    HB = BS // 2
                    nc.sync.dma_start(cT_d[d, hc, :, :HB], cnew[:, :HB])
                    nc.gpsimd.dma_start(cT_d[d, hc, :, HB:], cnew[:, HB:])
                else:
                    nc.scalar.dma_start(cT_d[d, hc], cnew[:])
                if isinstance(gts[2], list):
                    HB = BS // 2
                    store_eng = (nc.scalar, nc.sync)
                    for h2, oH in enumerate(gts[2]):
                        hnH = tmp_pool.tile([128, HB], F32,
                                            name="hnH", tag="hnew")
                        nc.vector.tensor_mul(
                            hnH[:], oH[:], tanhc[:, h2 * HB:(h2 + 1) * HB])
                        store_eng[h2].dma_start(
                            hT_d[d, hc, :, h2 * HB:(h2 + 1) * HB], hnH[:])
                else:
                    hnew = tmp_pool.tile([128, BS], F32, name="hnew",
                                         tag="hnew")
                    nc.vector.tensor_mul(hnew[:], gts[2][:], tanhc[:])
                    nc.scalar.dma_start(hT_d[d, hc], hnew[:])

            def steady_group(d, hc, grp):
                """One full group, phase-major: all DoubleRow (fp8)
                matmuls first, then all fp16 ones. An fp16->DR weight
                transition costs ~0.5 us on the weight-load path (FWL
                vs DoubleRow XBUS config), so the group pays it once,
                not once per gate."""
                wt8, wt8x, wts = grp["wt8"], grp["wt8x"], grp["wts"]
                final = d == 1 and hc == HC - 1
                HB = BS // 2
                pss = {}
                # fp8 phase: four h-quads (o's in halves for the final
                # group), then the f gate's x-part.
                for gi, g in enumerate(GPERM):
                    if final and gi == 3:
                        pss[g] = [psum_pool.tile([128, HB], F32,
                                                 name="psH", tag="ps")
                                  for _ in range(2)]
                        for h2 in range(2):
                            for j in range(KH // 2):
                                nc.tensor.matmul(
                                    pss[g][h2][:],
                                    wt8[:, gi, 2 * j:2 * j + 2, :],
                                    state["combh"][:, 2 * j:2 * j + 2,
                                                   h2 * HB:(h2 + 1) * HB],
                                    start=(j == 0), stop=False,
                                    perf_mode=DR,
                                )
                        continue
                    pss[g] = psum_pool.tile([128, BS], F32, name="ps",
                                            tag="ps")
                    h_quad(pss[g], gi, wt8, start=True)
                fx_mms(pss[0], wt8x)
                gts = {0: act_gate(d, hc, 0, pss[0])}
                # fp16 phase.
                for g in (3, 1, 2):
                    if final and g == 2:
                        halves = []
                        for h2 in range(2):
                            for k in range(KX):
                                base = (k % 4) * BS + h2 * HB
                                nc.tensor.matmul(
                                    pss[g][h2][:], wts[g][:, ts(k, 128)],
                                    state["combxs"][k // 4][:,
                                                            base:base + HB],
                                    start=False, stop=(k == KX - 1),
                                )
                            gtH = gate_pool.tile([128, HB], F32,
                                                 name="gtH", tag="gt")
                            nc.scalar.activation(
                                gtH[:], pss[g][h2][:], AF.Sigmoid,
                                bias=state["bias_t"][:, g * HC + hc:
                                                     g * HC + hc + 1],
                            )
                            halves.append(gtH)
                        gts[g] = halves
                        continue
                    x_gate(pss[g], wts[g])
                    gts[g] = act_gate(d, hc, g, pss[g])
                gts = [gts[0], gts[1], gts[2], gts[3]]
                fuse_store(d, hc, gts, grp["ct"])

            for d in range(2):
                combh = comb_pool.tile([128, KH, BS], F8, name="combh",
                                       tag="combh")
                state["combh"] = combh
                if d == 0:
                    # three slices matching the j-major startup schedule.
                    nc.scalar.dma_start(combh[:, :2, :],
                                        combh_d[d, :, :2, :])
                    nc.scalar.dma_start(combh[:, 2:4, :],
                                        combh_d[d, :, 2:4, :])
                    nc.scalar.dma_start(combh[:, 4:, :],
                                        combh_d[d, :, 4:, :])
                else:
                    nc.scalar.dma_start(combh[:, :KH // 2, :],
                                        combh_d[d, :, :KH // 2, :])
                    nc.scalar.dma_start(combh[:, KH // 2:, :],
                                        combh_d[d, :, KH // 2:, :])
                combx8 = comb_pool.tile([128, KX, BS], F8, name="combx8",
                                        tag="combx8")
                state["combx8"] = combx8
                if d == 0:
                    nc.scalar.dma_start(combx8[:, :4, :],
                                        combx8_d[d, :, :4, :])
                    nc.scalar.dma_start(combx8[:, 4:, :],
                                        combx8_d[d, :, 4:, :])
                else:
                    nc.gpsimd.dma_start(combx8[:], combx8_d[d])
                bias_t = bias_pool.tile([128, 4 * HC], F32, name="bias_t",
                                        tag="bias_t")
                state["bias_t"] = bias_t
                nc.gpsimd.dma_start(bias_t[:], bias_d[d])
                combxs = []
                for cc in range(2):
                    cb = comb_pool.tile([128, 4 * BS], F16,
                                        name=f"combx{cc}", tag=f"combx{cc}")
                    if d == 0 and cc == 0:
                        nc.gpsimd.dma_start(cb[:, :BS],
                                            combx_d[d, :, :BS])
                        nc.scalar.dma_start(cb[:, BS:],
                                            combx_d[d, :, BS:4 * BS])
                    elif d == 0:
                        pass  # issued inside load_group(0, 0)
                    else:
                        nc.scalar.dma_start(cb[:],
                                            combx_d[d, :, ts(cc, 4 * BS)])
                    combxs.append(cb)
                state["combxs"] = combxs

                if d == 0:
                    # ---- startup: groups 0 and 1 interleaved ----------
                    g0 = load_group(0, 0)
                    g1 = load_group(0, 1)
                    # h-parts of group 0, j-major with dummy seams.
                    pss0 = {g: psum_pool.tile([128, BS], F32, name="ps",
                                              tag="ps") for g in GPERM}
                    for j in range(KH // 2):
                        for gi, g in enumerate(GPERM):
                            nc.tensor.matmul(
                                pss0[g][:],
                                g0["wt8"][:, gi, 2 * j:2 * j + 2, :],
                                combh[:, 2 * j:2 * j + 2, :],
                                start=(j == 0), stop=False, perf_mode=DR,
                            )
                        if j == 0:
                            dummy_mms(2)
                        elif j == 1:
                            dummy_mms(4)
                    # f gate x-part of group 0 (fp8, arrives early).
                    fx_mms(pss0[0], g0["wt8x"], seam=2)
                    # h-parts of group 1's C/f gates: more DMA-cheap
                    # work while the fp16 stream ramps (2 more open
                    # psum banks; i/o quads run after group 0 retires).
                    pss1 = {}
                    for gi, g in [(0, 3), (1, 0)]:
                        pss1[g] = psum_pool.tile([128, BS], F32,
                                                 name="ps", tag="ps")
                        h_quad(pss1[g], gi, g1["wt8"], start=True)
                    # group 0's fp16 x-phase, k-major.
                    x_kmajor(pss0, g0["wts"])
                    gts0 = {}
                    for g in (0, 3, 1, 2):
                        gts0[g] = act_gate(0, 0, g, pss0[g])
                    fuse_store(0, 0, [gts0[0], gts0[1], gts0[2], gts0[3]],
                               g0["ct"])
                    # group 1: remaining work.
                    fx_mms(pss1[0], g1["wt8x"])
                    for gi, g in [(2, 1), (3, 2)]:
                        pss1[g] = psum_pool.tile([128, BS], F32,
                                                 name="ps", tag="ps")
                        h_quad(pss1[g], gi, g1["wt8"], start=True)
                    x_kmajor(pss1, g1["wts"])
                    gts1 = {}
                    for g in (0, 3, 1, 2):
                        gts1[g] = act_gate(0, 1, g, pss1[g])
                    fuse_store(0, 1, [gts1[0], gts1[1], gts1[2], gts1[3]],
                               g1["ct"])
                    hc_start = 2
                else:
                    hc_start = 0
                for hc in range(hc_start, HC):
                    grp = load_group(d, hc)
                    steady_group(d, hc, grp)
    nc.compile()
    return nc


def _prep_w(W):
    # W [4, 1024, 2048] f32 (gate, h, i) -> (wx fp16, wx8 fp8, wh fp8):
    # wx  [HC, 4(perm), 128 i_local, KX*128 (k, h_local)] from i in [0, 1024)
    # wx8 [HC, 128 i_local, KX, 128 h_local]  f-gate slice of the same range
    # wh  [HC, 128 i_local, 4(perm), KH, 128 h_local]  from i in [1024, 2048)
    # so the lhsT tile for (gate, hc, k) has i on partitions, with the gate
    # dim pre-permuted to the kernel's consumption order.
    w5 = W.reshape(4, HC, 128, 16, 128).transpose(0, 1, 4, 3, 2)[list(GPERM)]
    # w5: [g(perm), hc, i_local, k(0..15), h_local]
    wx = np.ascontiguousarray(
        w5[:, :, :, :KX, :].transpose(1, 0, 2, 3, 4)
    ).astype(np.float16).reshape(HC, 4, 128, KX * 128)
    # f gate sits at permuted slot 1 (GPERM.index of gate 0)
    wx8 = np.ascontiguousarray(
        w5[1, :, :, :KX, :]
    ).astype(ml_dtypes.float8_e5m2)
    wh = np.ascontiguousarray(
        w5[:, :, :, KX:, :].transpose(1, 2, 0, 3, 4)
    ).astype(ml_dtypes.float8_e5m2)
    return wx, wx8, wh


def _prep_combx(x_slice):
    # [BS, 1024] f16 -> [128 i_local, KX*BS (k, b)]
    return np.ascontiguousarray(
        x_slice.T.reshape(KX, 128, BS).transpose(1, 0, 2)
    ).reshape(128, KX * BS)


def _prep_comb8(x_slice):
    # [BS, 1024] f32 -> fp8 [128 i_local, K, BS]
    return np.ascontiguousarray(
        x_slice.T.reshape(KX, 128, BS).transpose(1, 0, 2)
    ).astype(ml_dtypes.float8_e5m2)


def _prep_ct(c_slice):
    # [BS, 1024] f32 -> [HC, 128 h_local, BS]
    return np.ascontiguousarray(c_slice.T).reshape(HC, 128, BS)


def _prep_bias(b):
    # [4, 1024] f32 -> [128 h_local, 4*HC (g, hc)]
    return np.ascontiguousarray(
        b.reshape(4, HC, 128).transpose(2, 0, 1)
    ).reshape(128, 4 * HC)


def kernel(input_f, input_b, Hidden_State_f, Cell_State_f,
           Hidden_State_b, Cell_State_b, Wf, bf, Wb, bb):
    global LAST_RESULTS

    args = [np.asarray(a, dtype=np.float32) for a in (
        input_f, input_b, Hidden_State_f, Cell_State_f,
        Hidden_State_b, Cell_State_b, Wf, bf, Wb, bb)]
    (input_f, input_b, Hidden_State_f, Cell_State_f,
     Hidden_State_b, Cell_State_b, Wf, bf, Wb, bb) = args

    xf16 = input_f.astype(np.float16)
    xb16 = input_b.astype(np.float16)
    wxf, wx8f, whf = _prep_w(Wf)
    wxb, wx8b, whb = _prep_w(Wb)
    wx_all = np.stack([wxf, wxb])
    wx8_all = np.stack([wx8f, wx8b])
    wh_all = np.stack([whf, whb])
    bias_all = np.stack([_prep_bias(bf), _prep_bias(bb)])

    in_maps = []
    for c in range(NCORES):
        sl = slice(c * BS, (c + 1) * BS)
        in_maps.append({
            "combx": np.stack([_prep_combx(xf16[sl]), _prep_combx(xb16[sl])]),
            "combx8": np.stack([_prep_comb8(input_f[sl]),
                                _prep_comb8(input_b[sl])]),
            "combh": np.stack([_prep_comb8(Hidden_State_f[sl]),
                               _prep_comb8(Hidden_State_b[sl])]),
            "wx": wx_all,
            "wx8": wx8_all,
            "wh": wh_all,
            "ct": np.stack([_prep_ct(Cell_State_f[sl]),
                            _prep_ct(Cell_State_b[sl])]),
            "bias": bias_all,
        })

    nc = _build_nc()
    res = bass_utils.run_bass_kernel_spmd(nc, in_maps,
                                          core_ids=list(range(NCORES)))
    LAST_RESULTS = res

    h_f = np.empty((BATCH, HID), np.float32)
    c_f = np.empty((BATCH, HID), np.float32)
    h_b = np.empty((BATCH, HID), np.float32)
    c_b = np.empty((BATCH, HID), np.float32)
    for c in range(NCORES):
        sl = slice(c * BS, (c + 1) * BS)
        r = res.results[c]
        hT, cT = r["hT"], r["cT"]  # [2, HC, 128, BS] f32
        h_f[sl] = hT[0].reshape(HID, BS).T
        c_f[sl] = cT[0].reshape(HID, BS).T
        h_b[sl] = hT[1].reshape(HID, BS).T
        c_b[sl] = cT[1].reshape(HID, BS).T
    return h_f, c_f, h_b, c_b
